# revision 1
# baseline (speedup 1.0000x reference)
"""Trainium2 Bass kernel for nn_CSAModule_47768626266174.

Mathematical structure of the reference:

    S    = softmax(attn, axis=-1)                # [C, T, T]
    out  = base + sigma * einsum('bft,ct->bcf', inputs, S.mean(axis=-1))
    base = inputs.mean(-1)[:, None, :]

``S.mean(axis=-1)`` averages over the *same* axis the softmax normalizes,
so it is exactly 1/T for every (c, t) — independent of the attention
contents, the conv weights, and the labels.  Hence

    out[b, c, f] = (1 + sigma) * mean_t inputs[b, f, t]

for every class c.  This identity holds for all finite inputs (softmax is
shift-normalized, rows sum to 1), so the kernel only needs to read
``inputs`` once, reduce over T, scale by (1 + sigma), and broadcast over
the class dim.  That is the true memory roofline of this module.

Sharding: data-parallel over batch B — each of the 8 cores reduces its
8-item chunk; no collectives.  Output chunks are concatenated on host.

Raw Bass (not Tile): this container's walrus build encodes at most ONE
semaphore wait per instruction, which rejects Tile's kernel-tail drain.
Standalone wait_ge instructions carry one condition each; anything
needing several predecessors gets several wait_ge's in front.

Per-core dataflow.  The critical path is the serialized DMA data stream
(~2.1 MB at ~360 GB/s); every other stage is pipelined per batch item
behind it, so the post-stream tail is just the last item's short chain:

  sync  : per-item input DMAs (per-DMA semaphores — dynamic HW queues
          complete out of order; the last item is loaded in two halves so
          the tail reduce is half-size), one store of y as [C, B, F]
  gpsimd: sigma DMA on SWDGE (keeps the HWDGE path free for x),
          ident_s = diag((1+sigma)/T) in one affine_select — all early
  vector: s1 = (1+sigma)/T, T-reduces (594 ns/item < 728 ns arrival),
          the last item's second-half reduce and PSUM->SBUF copy
  scalar: the last item's FIRST-half reduce (activation w/ accum_out,
          slotted between copies) so DVE reaches the critical final
          half-reduce with no backlog; per finished item, the PSUM ->
          SBUF copy of its y rows into yt [C, B*F] (compute engines can
          only address partition bases 0/32/64/96, so items advance
          along the free axis)
  tensor: K=1 matmul broadcasting s1 across partitions into psc; then per
          item b a small matmul into its own PSUM bank:
          pt_b = (sums[:, b] bcast over C).T @ ident_s  ([C, F] = y rows)
          (the tail item as two PSUM-accumulating half matmuls)
"""

from contextlib import ExitStack

import numpy as np

B, F, T, C = 64, 128, 512, 10
N_CORES = 8
BPC = B // N_CORES  # batch items per core

_NC_CACHE = None


def _build_bass():
    """Build the per-core Bass module (SPMD: same program on all cores)."""
    global _NC_CACHE
    if _NC_CACHE is not None:
        return _NC_CACHE

    import concourse.bass as bass
    import concourse.mybir as mybir

    fp32 = mybir.dt.float32
    # Bass.__init__ unconditionally memsets four const-AP tiles on the
    # Pool engine BEFORE the program start barrier; Pool is the last
    # engine to reach that barrier, so they delay every engine's release
    # by ~0.3 us.  None of them have readers in this kernel (walrus flags
    # them as dead), so skip their emission.  memset is re-bound into
    # BassEitherVectorEngine at class-definition time — patch there.
    _orig_memset = bass.BassEitherVectorEngine.memset

    def _memset_skip_dead_consts(self, ap, constant):
        tensor = getattr(ap, "tensor", None)
        if tensor is not None and getattr(tensor, "name", "").startswith(
            "const-"
        ):
            return None
        return _orig_memset(self, ap, constant)

    # The start barrier emitted at the end of Bass.__init__ only orders
    # those const-tile memsets against the program body; with the memsets
    # gone, every cross-engine dependency in this kernel is already
    # semaphore-guarded (CoreSim's race detector verifies), so skip it
    # too — it costs ~0.5 us before the first DMA can issue.
    _orig_barrier = bass.Bass.all_engine_barrier

    def _skip_barrier(self, *, sem_only: bool = False):
        return None

    bass.BassEitherVectorEngine.memset = _memset_skip_dead_consts
    bass.Bass.all_engine_barrier = _skip_barrier
    try:
        nc = bass.Bass()
    finally:
        bass.BassEitherVectorEngine.memset = _orig_memset
        bass.Bass.all_engine_barrier = _orig_barrier

    x = nc.dram_tensor("x", [BPC, F, T], fp32, kind="ExternalInput")
    sig = nc.dram_tensor("sig", [1, 1], fp32, kind="ExternalInput")
    y = nc.dram_tensor("y", [BPC, C, F], fp32, kind="ExternalOutput")

    with ExitStack() as ctx:
        e = ctx.enter_context
        xt = e(nc.sbuf_tensor("xt", [128, BPC * T], fp32))
        # SPLIT items are loaded/reduced in two halves so the tail reduce
        # is half-size and starts earlier; the PE recombines each pair via
        # PSUM accumulation.  Only the last item: each dma_start costs
        # ~650 ns of issue time vs 364 ns of data time for a half chunk,
        # so more splits stall the stream on descriptor generation.
        SPLIT = [BPC - 1]
        # A few spare columns for split items' partial sums.
        sums = e(nc.sbuf_tensor("sums", [128, BPC + 4], fp32))
        ident_s = e(nc.sbuf_tensor("ident_s", [128, 128], fp32))
        sg = e(nc.sbuf_tensor("sg", [1, 1], fp32))
        s1 = e(nc.sbuf_tensor("s1", [1, 1], fp32))
        ones_row = e(nc.sbuf_tensor("ones_row", [1, 128], fp32))
        scale_col = e(nc.sbuf_tensor("scale_col", [128, 1], fp32))
        # [C partitions, BPC*F free]: per-item copies land at free-dim
        # offsets (compute engines may only start at partition 0/32/64/96).
        yt = e(nc.sbuf_tensor("yt", [C, BPC * F], fp32))
        # psc is allocated and immediately freed: its bank is reused by
        # pts[0].  Safe because the first per-item matmul waits for the
        # scale_col copy, after which psc is dead.
        psc_cm = nc.psum_tensor("psc", [128, 1], fp32)
        psc = psc_cm.__enter__()
        psc_cm.__exit__(None, None, None)
        # One PSUM bank per item: matmul outputs must start at partition
        # 0/32/64, and bank separation means the PE write of item b+1
        # never touches the bank ACT is reading for item b.
        pts = [e(nc.psum_tensor(f"pt{b}", [C, 128], fp32)) for b in range(BPC)]

        # Load plan: (item, sums-column, t-range, semaphore, reduce
        # engine).  One semaphore per DMA: dynamic HW queues complete out
        # of order.  The split item's FIRST half reduces on ACT, so DVE
        # reaches the critical final half-reduce with no backlog.  Two
        # chunks measured best: each extra chunk adds a full matmul
        # (~213 ns) to the serial PE tail but saves only ~146 ns of
        # reduce time.
        H = T // 2
        TAIL_CHUNKS = [(0, H, "act"), (H, T, "dve")]
        loads = []
        extra_col = BPC
        for b in range(BPC):
            if b in SPLIT:
                for i, (t0, t1, eng) in enumerate(TAIL_CHUNKS):
                    col = b if i == 0 else extra_col
                    if i > 0:
                        extra_col += 1
                    loads.append(
                        (b, col, t0, t1, e(nc.semaphore(f"xld{b}_{i}")), eng)
                    )
            else:
                loads.append((b, b, 0, T, e(nc.semaphore(f"xld{b}")), "dve"))

        dump = e(nc.sbuf_tensor("dump", [128, H], fp32))

        sig_sem = e(nc.semaphore("sig_sem"))
        store_sem = e(nc.semaphore("store_sem"))
        dve_sem = e(nc.semaphore("dve_sem"))
        act_sem = e(nc.semaphore("act_sem"))
        act_red_sem = e(nc.semaphore("act_red_sem"))
        pe_sem = e(nc.semaphore("pe_sem"))
        pool_sem = e(nc.semaphore("pool_sem"))

        block = e(nc.Block())

        # Precomputed milestones (block bodies trace immediately, so no
        # cross-block mutable state).
        # dve_sem: 1 ones_row, 2 s1, 3 scale_col, then one per DVE reduce
        # in load order, then +1 for the last item's PSUM->SBUF copy.
        dve_red_ms = {}
        act_red_ms = {}
        dve_n = 3
        act_red_n = 0
        for b, col, t0, t1, sem, eng in loads:
            if eng == "dve":
                dve_n += 1
                dve_red_ms[(b, col)] = dve_n
            else:
                act_red_n += 1
                act_red_ms[(b, col)] = act_red_n
        DVE_COPY_MS = dve_n + 1
        # pe_sem: 1 psc, then one per matmul in item order (split item's
        # halves are consecutive accumulating matmuls).
        mm_plan = []  # (item, col, start, stop, wait_engine, wait_value)
        pe_n = 1
        mm_ms = {}
        for b in range(BPC):
            cols = [(col, eng) for (bb, col, t0, t1, s, eng) in loads if bb == b]
            for i, (col, eng) in enumerate(cols):
                wait = (
                    ("dve", dve_red_ms[(b, col)])
                    if eng == "dve"
                    else ("act", act_red_ms[(b, col)])
                )
                pe_n += 1
                mm_plan.append(
                    (b, col, i == 0, i == len(cols) - 1, wait[0], wait[1])
                )
            mm_ms[b] = pe_n

        @block.sync
        def _(sync):
            for b, col, t0, t1, sem, eng in loads:
                sync.dma_start(
                    xt[:, b * T + t0 : b * T + t1], x[b, :, t0:t1]
                ).then_inc(sem, 16)
            sync.wait_ge(act_sem, BPC - 1)  # yt columns 0..BPC-2 copied
            sync.wait_ge(dve_sem, DVE_COPY_MS)  # last yt column (DVE)
            sync.dma_start(
                y[:, :, :].rearrange("b c f -> c b f"),
                yt[:, :].rearrange("c (b f) -> c b f", f=F),
            ).then_inc(store_sem, 16)
            sync.wait_ge(store_sem, 16)

        @block.gpsimd
        def _(gpsimd):
            # SWDGE load of sigma — the HWDGE descriptor path stays free
            # for the x stream.
            gpsimd.dma_start(sg[:, :], sig[:, :]).then_inc(sig_sem, 16)
            # ident_s = diag((1+sigma)/T) in a single op: select between a
            # step-0 broadcast of scale_col and 0.0.
            gpsimd.wait_ge(dve_sem, 3)  # scale_col ready
            gpsimd.affine_select(
                out=ident_s[:, :],
                in_=scale_col[:, :].broadcast_to((128, 128)),
                compare_op=mybir.AluOpType.is_equal,
                fill=0.0,
                base=0,
                pattern=[[-1, 128]],
                channel_multiplier=1,
            ).then_inc(pool_sem, 1)  # p1

        @block.vector
        def _(vector):
            vector.memset(ones_row[:, :], 1.0).then_inc(dve_sem, 1)
            vector.wait_ge(sig_sem, 16)
            # s1 = sigma/T + 1/T = (1+sigma)/T
            vector.tensor_scalar(
                out=s1[:, :],
                in0=sg[:, :],
                scalar1=1.0 / T,
                scalar2=1.0 / T,
                op0=mybir.AluOpType.mult,
                op1=mybir.AluOpType.add,
            ).then_inc(dve_sem, 1)
            vector.wait_ge(pe_sem, 1)  # psc ready
            vector.tensor_copy(scale_col[:, :], psc[:, :]).then_inc(dve_sem, 1)
            for b, col, t0, t1, sem, eng in loads:
                if eng != "dve":
                    continue
                vector.wait_ge(sem, 16)
                vector.reduce_sum(
                    out=sums[:, col : col + 1],
                    in_=xt[:, b * T + t0 : b * T + t1],
                    axis=mybir.AxisListType.X,
                ).then_inc(dve_sem, 1)
            # Last item's PSUM -> SBUF copy on DVE (free after its final
            # reduce, and its copy is faster than ACT's).
            vector.wait_ge(pe_sem, mm_ms[BPC - 1])
            vector.tensor_copy(
                yt[:, (BPC - 1) * F : BPC * F], pts[BPC - 1][:, :]
            ).then_inc(dve_sem, 1)

        @block.tensor
        def _(tensor):
            tensor.wait_ge(dve_sem, 2)  # ones_row + s1
            # psc[p, 0] = (1+sigma)/T on every partition (K=1 matmul).
            tensor.matmul(
                psc[:, :], ones_row[:, :], s1[:, :], start=True, stop=True
            ).then_inc(pe_sem, 1)
            tensor.wait_ge(pool_sem, 1)  # ident_s ready
            # Per-item matmuls, issued as each (partial) reduce lands:
            # pt_b[c, f] = sums[f, b] * (1+sigma)/T.  lhsT is the item's
            # sums column broadcast over classes via one step-0 free dim;
            # the sigma scale rides the diagonal matrix; split items
            # accumulate their halves in PSUM.
            for b, col, is_start, is_stop, weng, wval in mm_plan:
                if weng == "dve":
                    tensor.wait_ge(dve_sem, wval)
                else:
                    tensor.wait_ge(act_red_sem, wval)
                tensor.matmul(
                    pts[b][:, :],
                    sums[:, col : col + 1].broadcast_to((128, C)),
                    ident_s[:, :],
                    start=is_start,
                    stop=is_stop,
                ).then_inc(pe_sem, 1)

        @block.scalar
        def _(scalar):
            # Per-item PSUM -> SBUF copies on the otherwise idle ACT
            # engine (the last item's copy runs on DVE instead), with the
            # split item's first-half reduce slotted in between: it must
            # come after enough copies that they are not delayed, but
            # before ACT goes idle waiting on late matmuls.
            act_loads = [
                ld for ld in loads if ld[5] == "act"
            ]
            for b in range(BPC - 1):
                if b == BPC - 3:
                    for bb, col, t0, t1, sem, eng in act_loads:
                        scalar.wait_ge(sem, 16)
                        scalar.activation(
                            out=dump[:, :],
                            in_=xt[:, bb * T + t0 : bb * T + t1],
                            func=mybir.ActivationFunctionType.Copy,
                            accum_out=sums[:, col : col + 1],
                        ).then_inc(act_red_sem, 1)
                scalar.wait_ge(pe_sem, mm_ms[b])
                scalar.activation(
                    out=yt[:, b * F : (b + 1) * F],
                    in_=pts[b][:, :],
                    func=mybir.ActivationFunctionType.Copy,
                ).then_inc(act_sem, 1)

    _NC_CACHE = nc
    return nc


def run_spmd(inputs_arr: np.ndarray, sigma_arr: np.ndarray, trace: bool = False):
    """Shard over batch, run on 8 cores, gather. Returns (out, results_obj)."""
    from concourse import bass_utils

    nc = _build_bass()

    x_full = np.ascontiguousarray(np.asarray(inputs_arr, dtype=np.float32))
    assert x_full.shape == (B, F, T), x_full.shape
    sig = np.asarray(sigma_arr, dtype=np.float32).reshape(1, 1)

    in_maps = [
        {"x": x_full[k * BPC : (k + 1) * BPC], "sig": sig} for k in range(N_CORES)
    ]
    res = bass_utils.run_bass_kernel_spmd(
        nc, in_maps, core_ids=list(range(N_CORES)), trace=trace
    )
    out = np.concatenate([r["y"] for r in res.results], axis=0)
    return out, res


def kernel(**inputs) -> np.ndarray:
    out, _ = run_spmd(inputs["inputs"], inputs["sigma"])
    return out



# revision 25
# speedup vs baseline: 1.1085x; 1.1085x over previous
"""Trainium2 Bass kernel for nn_CSAModule_47768626266174.

Mathematical structure of the reference:

    S    = softmax(attn, axis=-1)                # [C, T, T]
    out  = base + sigma * einsum('bft,ct->bcf', inputs, S.mean(axis=-1))
    base = inputs.mean(-1)[:, None, :]

``S.mean(axis=-1)`` averages over the *same* axis the softmax normalizes,
so it is exactly 1/T for every (c, t) — independent of the attention
contents, the conv weights, and the labels.  Hence

    out[b, c, f] = (1 + sigma) * mean_t inputs[b, f, t]

for every class c.  The kernel reads ``inputs`` once, reduces over T,
scales by (1 + sigma)/T, and broadcasts over the class dim.

Sharding: data-parallel over batch B — each of the 8 cores reduces its
8-item chunk; no collectives.  Output chunks are concatenated on host.

Raw Bass (not Tile): this container's walrus build encodes at most ONE
semaphore wait per instruction; standalone wait_ge instructions carry
one condition each.  Walrus also limits the matmul stationary operand to
one free dimension, so multi-item transpose inputs are materialized by
per-item broadcast copies instead of multi-dim broadcast APs.

Per-core dataflow.  The serialized DMA input stream (~2 MB at 360 GB/s,
ends ~7.4 us) is the floor; everything else hides behind it except the
last chunks' dependency chains:

  * items 0..6 stream as full [128, 512] chunks; item 7 streams as two
    256-col chunks so its two DVE reduces overlap the arrival sems (the
    first chunk's partial sum lands in xt[:, 4096] and the tail reduce
    is one contiguous [128, 257] op).
  * the f(partition) -> f(free) flip with class broadcast is a PE
    transpose-mode matmul (2 cycles/row vs 4 for fp32 matmul) against a
    0/1 identity.  Dummy K=1 matmuls paced by the per-item reduces keep
    the PE p-state ramped so the tail transposes run at full clock.
  * the sigma scale rides the PSUM -> SBUF copies (ACT activation scale
    / DVE multiply) via scale_col = (1+sigma)/T on every partition.
  * the store is three SWDGE dma_scatter_add's PREPARED early on Pool
    and fired by trigger_dma when their source rows are ready: a
    triggered SWDGE transfer skips both the ~650 ns HWDGE issue and the
    ~650 ns DGE->DMA delay that a dma_start pays after its wait.
      A  = items 0..5 (60 rows) — everything hidden in the stream;
           fires as soon as the zero-store completes.
      B6 = item 6 (10 rows) — reduce on ACT, transpose, ACT copy.
      B7 = item 7 (10 rows) — reduces on DVE, transpose, DVE copy;
           this is the critical tail: last-chunk sem -> 257-col reduce
           -> transpose -> copy -> trigger -> 14 ns transfer -> DMA sem.
    Item 6's chain (ACT) and item 7's chain (DVE) share no engine, so
    the two post-stream dependency chains run in parallel.
  * scatter_add accumulates into HBM, so y is pre-zeroed by a plain
    store of zeros issued on SP after the 9 input loads — its transfer
    runs in the post-stream gap and completes before any trigger fires.
"""

from contextlib import ExitStack

import numpy as np

B, F, T, C = 64, 128, 512, 10
N_CORES = 8
BPC = B // N_CORES  # batch items per core

MAIN = 256  # item 7 first-chunk columns
TAIL = T - MAIN  # item 7 tail-chunk columns

_NC_CACHE = None


def _build_bass():
    """Build the per-core Bass module (SPMD: same program on all cores)."""
    global _NC_CACHE
    if _NC_CACHE is not None:
        return _NC_CACHE

    import concourse.bass as bass
    import concourse.bass_isa as bass_isa
    import concourse.mybir as mybir
    from concourse import library_config

    fp32 = mybir.dt.float32
    i16 = mybir.dt.int16

    # Raw Bass skips Bacc's codegen_inst_isa_subclasses pass, which
    # populates the encoded .instr bytes of extended-inst InstISA
    # subclasses (trigger_dma, the load_library MPC, ...).  Without it
    # the NEFF compiler sees empty .instr -> "ISA wrong length", and
    # hand-rolled encodings mismatch the device firmware.  The pass is
    # run on the finished module just before caching (see below, after
    # the blocks are traced).

    # Bass.__init__ unconditionally memsets four const-AP tiles on the
    # Pool engine BEFORE the program start barrier; none have readers in
    # this kernel, so skip their emission (and the start barrier, whose
    # only job was ordering them) — saves ~0.8 us of head latency.
    _orig_memset = bass.BassEitherVectorEngine.memset

    def _memset_skip_dead_consts(self, ap, constant):
        tensor = getattr(ap, "tensor", None)
        if tensor is not None and getattr(tensor, "name", "").startswith(
            "const-"
        ):
            return None
        return _orig_memset(self, ap, constant)

    _orig_barrier = bass.Bass.all_engine_barrier

    def _skip_barrier(self, *, sem_only: bool = False):
        return None

    bass.BassEitherVectorEngine.memset = _memset_skip_dead_consts
    bass.Bass.all_engine_barrier = _skip_barrier
    try:
        nc = bass.Bass()
    finally:
        bass.BassEitherVectorEngine.memset = _orig_memset
        bass.Bass.all_engine_barrier = _orig_barrier

    x = nc.dram_tensor("x", [BPC, F, T], fp32, kind="ExternalInput")
    sig = nc.dram_tensor("sig", [1, 1], fp32, kind="ExternalInput")
    # Host-precomputed scatter token indices, replicated per 16-partition
    # Q7 group: cols 0..3 = scatter-A tokens, col 4 = scatter-B tokens.
    idx = nc.dram_tensor("idx", [128, 8], i16, kind="ExternalInput")
    y = nc.dram_tensor("y", [BPC, C, F], fp32, kind="ExternalOutput")

    NA = 6 * C  # scatter-A rows (items 0..5)
    NB = C  # scatter-B6 / B7 rows (one item each)
    yrows = y.rearrange("b c f -> (b c) f")

    with ExitStack() as ctx:
        e = ctx.enter_context
        # xt column 4096 holds item 7's first-chunk partial sum so the
        # tail reduce is one contiguous [128, TAIL+1] op.
        xt = e(nc.sbuf_tensor("xt", [128, BPC * T + 4], fp32))
        sums = e(nc.sbuf_tensor("sums", [128, BPC], fp32))
        lhsTA = e(nc.sbuf_tensor("lhsTA", [128, NA], fp32))
        sg = e(nc.sbuf_tensor("sg", [1, 1], fp32))
        s1 = e(nc.sbuf_tensor("s1", [1, 1], fp32))
        ones_row = e(nc.sbuf_tensor("ones_row", [1, 128], fp32))
        ones_col = e(nc.sbuf_tensor("ones_col", [128, 1], fp32))
        scale_col = e(nc.sbuf_tensor("scale_col", [128, 1], fp32))
        ident01 = e(nc.sbuf_tensor("ident01", [128, 128], fp32))
        # ys[:, 0:128]   scatter-A rows (partitions 0..59)
        # ys[:, 128:256] scatter-B6 rows (partitions 0..9)
        # ys[:, 256:384] scatter-B7 rows (partitions 0..9)
        # ys[:, 0:80]    zero source for the y pre-zero store
        ys = e(nc.sbuf_tensor("ys", [128, 384], fp32))
        dump = e(nc.sbuf_tensor("dump", [128, T], fp32))
        idx_sb = e(nc.sbuf_tensor("idx_sb", [128, 8], i16))

        psc = e(nc.psum_tensor("psc", [128, 1], fp32))
        ptsA = e(nc.psum_tensor("ptsA", [NA, 128], fp32))
        ptsB6 = e(nc.psum_tensor("ptsB6", [NB, 128], fp32))
        ptsB7 = e(nc.psum_tensor("ptsB7", [NB, 128], fp32))

        xld = [e(nc.semaphore(f"xld{k}")) for k in range(BPC + 1)]
        sig_sem = e(nc.semaphore("sig_sem"))
        idx_sem = e(nc.semaphore("idx_sem"))
        zero_sem = e(nc.semaphore("zero_sem"))
        prepA_sem = e(nc.semaphore("prepA_sem"))
        prepB6_sem = e(nc.semaphore("prepB6_sem"))
        prepB7_sem = e(nc.semaphore("prepB7_sem"))
        dmaA_sem = e(nc.semaphore("dmaA_sem"))
        dmaB6_sem = e(nc.semaphore("dmaB6_sem"))
        dmaB7_sem = e(nc.semaphore("dmaB7_sem"))
        dve_sem = e(nc.semaphore("dve_sem"))
        act_sem = e(nc.semaphore("act_sem"))
        pe_sem = e(nc.semaphore("pe_sem"))
        pool_sem = e(nc.semaphore("pool_sem"))

        block = e(nc.Block())

        # DVE milestones (then_inc order): 1 ones_col, 2 ones_row,
        # 3 ys memset, 4 s1, 5 scale_col, 6..11 items 0..5, 12 item7
        # main, 13 item7 tail, 14 copyB7.
        MS_YS = 3
        MS_ITEM = {b: 6 + b for b in range(6)}
        MS_TAIL7 = 13
        MS_COPYB7 = 14
        # ACT milestones: 1..6 lhsTA copies, 7 item6 reduce, 8 copyA,
        # 9 copyB6.
        # PE milestones: 1 psc, 2..7 dummies, 8 trA, 9 trB6, 10 trB7.

        @block.sync
        def _(sync):
            # Items 0..6 as full chunks, then item 7 as MAIN + TAIL.
            for b in range(BPC - 1):
                sync.dma_start(
                    xt[:, b * T : (b + 1) * T], x[b, :, :]
                ).then_inc(xld[b], 16)
            b = BPC - 1
            sync.dma_start(
                xt[:, b * T : b * T + MAIN], x[b, :, 0:MAIN]
            ).then_inc(xld[b], 16)
            sync.dma_start(
                xt[:, b * T + MAIN : (b + 1) * T], x[b, :, MAIN:T]
            ).then_inc(xld[BPC], 16)
            # Pre-zero y (scatter_add accumulates).  The zeros are read
            # from the still-zero ys region; the transfer lands right
            # after the input stream, its sem before any trigger fires.
            sync.wait_ge(dve_sem, MS_YS)
            sync.dma_start(
                y[:, :, :]
                .rearrange("b c f -> (b c f)")
                .rearrange("(p q) -> p q", p=128),
                ys[:, 0 : (BPC * C * F) // 128],
            ).then_inc(zero_sem, 16)
            sync.wait_ge(dmaA_sem, 16)
            sync.wait_ge(dmaB6_sem, 16)
            sync.wait_ge(dmaB7_sem, 16)

        @block.gpsimd
        def _(gpsimd):
            # SWDGE sigma + scatter-index loads keep HWDGE free for the
            # input stream.
            gpsimd.dma_start(sg[:, :], sig[:, :]).then_inc(sig_sem, 16)
            gpsimd.dma_start(idx_sb[:, :], idx[:, :]).then_inc(idx_sem, 16)
            # Prepared scatters; trigger_dma fires them in FIFO order
            # (A, then B6, then B7).  dma_scatter_add lives in the 'mlp'
            # Q7 ucode library.
            gpsimd.wait_ge(idx_sem, 16)
            gpsimd.load_library(library_config.mlp)
            gpsimd.dma_scatter_add(
                out_ap=yrows[0:NA, :],
                in_ap=ys[:, 0:128].rearrange("p (s e) -> p s e", s=1),
                idxs_ap=idx_sb[:, 0:4],
                num_idxs=NA,
                num_idxs_reg=NA,
                elem_size=128,
                prepare_only=True,
                sem=dmaA_sem,
            ).then_inc(prepA_sem, 1)
            gpsimd.dma_scatter_add(
                out_ap=yrows[NA : NA + NB, :],
                in_ap=ys[:, 128:256].rearrange("p (s e) -> p s e", s=1),
                idxs_ap=idx_sb[:, 4:5],
                num_idxs=NB,
                num_idxs_reg=NB,
                elem_size=128,
                prepare_only=True,
                sem=dmaB6_sem,
            ).then_inc(prepB6_sem, 1)
            gpsimd.dma_scatter_add(
                out_ap=yrows[NA + NB : NA + 2 * NB, :],
                in_ap=ys[:, 256:384].rearrange("p (s e) -> p s e", s=1),
                idxs_ap=idx_sb[:, 4:5],
                num_idxs=NB,
                num_idxs_reg=NB,
                elem_size=128,
                prepare_only=True,
                sem=dmaB7_sem,
            ).then_inc(prepB7_sem, 1)
            # 0/1 identity for the PE transposes.
            gpsimd.wait_ge(dve_sem, 1)  # ones_col
            gpsimd.affine_select(
                out=ident01[:, :],
                in_=ones_col[:, :].broadcast_to((128, 128)),
                compare_op=mybir.AluOpType.is_equal,
                fill=0.0,
                base=0,
                pattern=[[-1, 128]],
                channel_multiplier=1,
            ).then_inc(pool_sem, 1)  # ident01 milestone: pool_sem >= 1
            # Fire A once its rows are copied and y is zeroed.
            gpsimd.wait_ge(prepA_sem, 1)
            gpsimd.wait_ge(zero_sem, 16)
            gpsimd.wait_ge(act_sem, 8)  # copyA
            gpsimd.trigger_dma(count=1)
            # Fire B6 / B7 as their rows land.
            gpsimd.wait_ge(prepB6_sem, 1)
            gpsimd.wait_ge(act_sem, 9)  # copyB6
            gpsimd.trigger_dma(count=1)
            gpsimd.wait_ge(prepB7_sem, 1)
            gpsimd.wait_ge(dve_sem, MS_COPYB7)
            gpsimd.trigger_dma(count=1)

        @block.vector
        def _(vector):
            vector.memset(ones_col[:, :], 1.0).then_inc(dve_sem, 1)
            vector.memset(ones_row[:, :], 1.0).then_inc(dve_sem, 1)
            vector.memset(ys[:, :], 0.0).then_inc(dve_sem, 1)
            vector.wait_ge(sig_sem, 16)
            # s1 = sigma/T + 1/T = (1+sigma)/T
            vector.tensor_scalar(
                out=s1[:, :],
                in0=sg[:, :],
                scalar1=1.0 / T,
                scalar2=1.0 / T,
                op0=mybir.AluOpType.mult,
                op1=mybir.AluOpType.add,
            ).then_inc(dve_sem, 1)
            vector.wait_ge(pe_sem, 1)  # psc ready
            vector.tensor_copy(scale_col[:, :], psc[:, :]).then_inc(dve_sem, 1)
            # Items 0..5 full reduces (item 6 runs on ACT).
            for b in range(6):
                vector.wait_ge(xld[b], 16)
                vector.reduce_sum(
                    out=sums[:, b : b + 1],
                    in_=xt[:, b * T : (b + 1) * T],
                    axis=mybir.AxisListType.X,
                ).then_inc(dve_sem, 1)
            # Item 7 main-chunk partial -> xt[:, BPC*T] so the tail
            # reduce is one contiguous op over [tail chunk | partial].
            b = BPC - 1
            vector.wait_ge(xld[b], 16)
            vector.reduce_sum(
                out=xt[:, BPC * T : BPC * T + 1],
                in_=xt[:, b * T : b * T + MAIN],
                axis=mybir.AxisListType.X,
            ).then_inc(dve_sem, 1)
            vector.wait_ge(xld[BPC], 16)
            vector.wait_ge(dve_sem, 12)  # main-chunk partial committed
            vector.reduce_sum(
                out=sums[:, b : b + 1],
                in_=xt[:, b * T + MAIN : BPC * T + 1],
                axis=mybir.AxisListType.X,
            ).then_inc(dve_sem, 1)
            # Scaled PSUM -> SBUF copy of item 7's rows.
            vector.wait_ge(pe_sem, 10)  # trB7 done
            vector.tensor_tensor(
                out=ys[0:NB, 256:384],
                in0=ptsB7[:, :],
                in1=scale_col[0:NB, :].broadcast_to((NB, 128)),
                op=mybir.AluOpType.mult,
            ).then_inc(dve_sem, 1)

        @block.tensor
        def _(tensor):
            tensor.wait_ge(dve_sem, 4)  # ones_row + s1
            tensor.matmul(
                psc[:, :], ones_row[:, :], s1[:, :], start=True, stop=True
            ).then_inc(pe_sem, 1)
            # Dummy matmuls paced by the per-item reduces keep the PE
            # p-state ramp alive so the tail transposes run at full
            # clock (2.4 GHz after >3 us of busy history).
            for k in range(6):
                tensor.wait_ge(dve_sem, MS_ITEM[k])
                tensor.matmul(
                    psc[0:1, :],
                    ones_row[:, 0:1],
                    s1[:, :],
                    start=True,
                    stop=True,
                ).then_inc(pe_sem, 1)
            # Transpose A: materialized [128, 60] lhsT -> ptsA.
            tensor.wait_ge(act_sem, 6)  # lhsTA copies done
            tensor.wait_ge(pool_sem, 1)  # ident01
            tensor.transpose(
                ptsA[:, :], lhsTA[:, :], ident01[:, :]
            ).then_inc(pe_sem, 1)
            # Transpose B6: [128, 10] bcast of sums col 6 -> ptsB6.
            tensor.wait_ge(act_sem, 7)  # item 6 reduce (ACT)
            tensor.transpose(
                ptsB6[:, :],
                sums[:, 6:7].broadcast_to((128, NB)),
                ident01[:, :],
            ).then_inc(pe_sem, 1)
            # Transpose B7: [128, 10] bcast of sums col 7 -> ptsB7.
            tensor.wait_ge(dve_sem, MS_TAIL7)
            tensor.transpose(
                ptsB7[:, :],
                sums[:, BPC - 1 : BPC].broadcast_to((128, NB)),
                ident01[:, :],
            ).then_inc(pe_sem, 1)

        @block.scalar
        def _(scalar):
            # Per-item lhsTA blocks: column b broadcast to 10 columns,
            # paced by the DVE reduces — all hidden in the stream.
            for b in range(6):
                scalar.wait_ge(dve_sem, MS_ITEM[b])
                scalar.activation(
                    out=lhsTA[:, b * C : (b + 1) * C],
                    in_=sums[:, b : b + 1].broadcast_to((128, C)),
                    func=mybir.ActivationFunctionType.Copy,
                ).then_inc(act_sem, 1)
            # Item 6 reduce on ACT (activation accum_out) so DVE is
            # free for item 7's two tail reduces.
            b = 6
            scalar.wait_ge(xld[b], 16)
            scalar.activation(
                out=dump[:, :],
                in_=xt[:, b * T : (b + 1) * T],
                func=mybir.ActivationFunctionType.Copy,
                accum_out=sums[:, b : b + 1],
            ).then_inc(act_sem, 1)
            # Scaled copies (scale rides the activation).
            scalar.wait_ge(pe_sem, 8)  # trA done
            scalar.wait_ge(zero_sem, 16)  # zero store has read ys
            scalar.activation(
                out=ys[0:NA, 0:128],
                in_=ptsA[:, :],
                func=mybir.ActivationFunctionType.Copy,
                scale=scale_col[0:NA, :],
            ).then_inc(act_sem, 1)
            scalar.wait_ge(pe_sem, 9)  # trB6 done
            scalar.activation(
                out=ys[0:NB, 128:256],
                in_=ptsB6[:, :],
                func=mybir.ActivationFunctionType.Copy,
                scale=scale_col[0:NB, :],
            ).then_inc(act_sem, 1)

    mybir.codegen_inst_isa_subclasses(nc)
    _NC_CACHE = nc
    return nc


def _make_idx() -> np.ndarray:
    """Scatter token indices, replicated per 16-partition Q7 group.

    Cols 0..3: scatter-A tokens (token 16*s + i -> local y row, identity
    for the first 60 tokens, 0-clamped padding after).  Col 4: scatter-B
    tokens (identity for the first 10).  Values are row offsets local to
    each scatter's out_ap.
    """
    idx = np.zeros((128, 8), dtype=np.int16)
    a = np.arange(64).reshape(4, 16).T  # [i, s] = 16*s + i
    a = np.where(a < 60, a, 0).astype(np.int16)
    b = np.arange(16, dtype=np.int16)
    b = np.where(b < 10, b, 0).astype(np.int16)
    for g in range(8):
        idx[16 * g : 16 * (g + 1), 0:4] = a
        idx[16 * g : 16 * (g + 1), 4] = b
    return idx


def run_spmd(inputs_arr: np.ndarray, sigma_arr: np.ndarray, trace: bool = False):
    """Shard over batch, run on 8 cores, gather. Returns (out, results_obj)."""
    from concourse import bass_utils

    nc = _build_bass()

    x_full = np.ascontiguousarray(np.asarray(inputs_arr, dtype=np.float32))
    assert x_full.shape == (B, F, T), x_full.shape
    sig = np.asarray(sigma_arr, dtype=np.float32).reshape(1, 1)
    idx = _make_idx()

    in_maps = [
        {"x": x_full[k * BPC : (k + 1) * BPC], "sig": sig, "idx": idx}
        for k in range(N_CORES)
    ]
    res = bass_utils.run_bass_kernel_spmd(
        nc, in_maps, core_ids=list(range(N_CORES)), trace=trace
    )
    out = np.concatenate([r["y"] for r in res.results], axis=0)
    return out, res


def kernel(**inputs) -> np.ndarray:
    out, _ = run_spmd(inputs["inputs"], inputs["sigma"])
    return out


# revision 54
# speedup vs baseline: 1.1437x; 1.0318x over previous
"""Trainium2 Bass kernel for nn_CSAModule_47768626266174.

Mathematical structure of the reference:

    S    = softmax(attn, axis=-1)                # [C, T, T]
    out  = base + sigma * einsum('bft,ct->bcf', inputs, S.mean(axis=-1))
    base = inputs.mean(-1)[:, None, :]

``S.mean(axis=-1)`` averages over the *same* axis the softmax normalizes,
so it is exactly 1/T for every (c, t) — independent of the attention
contents, the conv weights, and the labels.  Hence

    out[b, c, f] = (1 + sigma) * mean_t inputs[b, f, t]

for every class c.  The kernel reads ``inputs`` once, reduces over T,
scales by (1 + sigma)/T, and broadcasts over the class dim.

Sharding: data-parallel over batch B — each of the 8 cores reduces its
8-item chunk; no collectives.  Output chunks are concatenated on host.

Raw Bass (not Tile): this container's walrus build encodes at most ONE
semaphore wait per instruction; standalone wait_ge instructions carry
one condition each.  Walrus also limits the matmul stationary operand to
one free dimension, so multi-item transpose inputs are materialized by
per-item broadcast copies instead of multi-dim broadcast APs.

Per-core dataflow.  The serialized DMA input stream (~2 MB at 360 GB/s,
ends ~7.4 us) is the floor; everything else hides behind it except the
last chunks' dependency chains:

  * items 0..6 stream as full [128, 512] chunks; item 7 streams as two
    256-col chunks so its two DVE reduces overlap the arrival sems (the
    first chunk's partial sum lands in xt[:, 4096] and the tail reduce
    is one contiguous [128, 257] op).
  * the f(partition) -> f(free) flip with class broadcast is a PE
    transpose-mode matmul (2 cycles/row vs 4 for fp32 matmul) against a
    0/1 identity.  Dummy K=1 matmuls paced by the per-item reduces keep
    the PE p-state ramped so the tail transposes run at full clock.
  * the sigma scale rides the PSUM -> SBUF copies (ACT activation scale
    / DVE multiply) via scale_col = (1+sigma)/T on every partition.
  * the store is three SWDGE dma_scatter_add's PREPARED early on Pool
    and fired by trigger_dma when their source rows are ready: a
    triggered SWDGE transfer skips both the ~650 ns HWDGE issue and the
    ~650 ns DGE->DMA delay that a dma_start pays after its wait.
      A  = items 0..5 (60 rows) — everything hidden in the stream;
           fires as soon as the zero-store completes.
      B6 = item 6 (10 rows) — reduce on ACT, transpose, ACT copy.
      B7 = item 7 (10 rows) — reduces on DVE, transpose, DVE copy;
           this is the critical tail: last-chunk sem -> 257-col reduce
           -> transpose -> copy -> trigger -> 14 ns transfer -> DMA sem.
    Item 6's chain (ACT) and item 7's chain (DVE) share no engine, so
    the two post-stream dependency chains run in parallel.
  * scatter_add accumulates into HBM, so y is pre-zeroed by a plain
    store of zeros issued on SP after the 9 input loads — its transfer
    runs in the post-stream gap and completes before any trigger fires.
"""

from contextlib import ExitStack

import numpy as np

B, F, T, C = 64, 128, 512, 10
N_CORES = 8
BPC = B // N_CORES  # batch items per core

MAIN = 256  # item 7 first-chunk columns
TAIL = T - MAIN  # item 7 tail-chunk columns

_NC_CACHE = None


def _build_bass():
    """Build the per-core Bass module (SPMD: same program on all cores)."""
    global _NC_CACHE
    if _NC_CACHE is not None:
        return _NC_CACHE

    import concourse.bass as bass
    import concourse.bass_isa as bass_isa
    import concourse.mybir as mybir
    from concourse import library_config

    fp32 = mybir.dt.float32
    i16 = mybir.dt.int16

    # Raw Bass skips Bacc's codegen_inst_isa_subclasses pass, which
    # populates the encoded .instr bytes of extended-inst InstISA
    # subclasses (trigger_dma, the load_library MPC, ...).  Without it
    # the NEFF compiler sees empty .instr -> "ISA wrong length", and
    # hand-rolled encodings mismatch the device firmware.  The pass is
    # run on the finished module just before caching (see below, after
    # the blocks are traced).

    # Bass.__init__ unconditionally memsets four const-AP tiles on the
    # Pool engine BEFORE the program start barrier; none have readers in
    # this kernel, so skip their emission (and the start barrier, whose
    # only job was ordering them) — saves ~0.8 us of head latency.
    _orig_memset = bass.BassEitherVectorEngine.memset

    def _memset_skip_dead_consts(self, ap, constant):
        tensor = getattr(ap, "tensor", None)
        if tensor is not None and getattr(tensor, "name", "").startswith(
            "const-"
        ):
            return None
        return _orig_memset(self, ap, constant)

    _orig_barrier = bass.Bass.all_engine_barrier

    def _skip_barrier(self, *, sem_only: bool = False):
        return None

    # End-of-program: the full barrier costs ~210 ns after the last DMA
    # sem (two sem-propagation rounds).  Emit only the InstDrains (which
    # carry the SWDGE sem-range resets) and let each engine halt
    # independently — the runtime waits for every engine's halt anyway,
    # and SP halts only after the scatter-completion sems.
    def _drains_only_barrier(self, *, sem_only: bool = False):
        for inst in self._all_engine_barrier_insts():
            if isinstance(inst, mybir.InstDrain):
                self.engines[inst.engine].add_instruction(inst)

    bass.BassEitherVectorEngine.memset = _memset_skip_dead_consts
    bass.Bass.all_engine_barrier = _skip_barrier
    try:
        nc = bass.Bass()
    finally:
        bass.BassEitherVectorEngine.memset = _orig_memset
    # Active through the Block exit below; restored after tracing.
    bass.Bass.all_engine_barrier = _drains_only_barrier

    x = nc.dram_tensor("x", [BPC, F, T], fp32, kind="ExternalInput")
    sig = nc.dram_tensor("sig", [1, 1], fp32, kind="ExternalInput")
    # Host-precomputed scatter token indices, replicated per 16-partition
    # Q7 group: cols 0..3 = scatter-A tokens, col 4 = scatter-B tokens.
    idx = nc.dram_tensor("idx", [128, 8], i16, kind="ExternalInput")
    y = nc.dram_tensor("y", [BPC, C, F], fp32, kind="ExternalOutput")

    NA = 6 * C  # scatter-A rows (items 0..5)
    NB = C  # scatter-B6 / B7 rows (one item each)
    yrows = y.rearrange("b c f -> (b c) f")

    with ExitStack() as ctx:
        e = ctx.enter_context
        # xt column 4096 holds item 7's first-chunk partial sum so the
        # tail reduce is one contiguous [128, TAIL+1] op.
        xt = e(nc.sbuf_tensor("xt", [128, BPC * T + 4], fp32))
        # sums cols: 0..5 items 0..5, 6 item 6, 8/9 item 7 chunk partials
        sums = e(nc.sbuf_tensor("sums", [128, 12], fp32))
        lhsTA = e(nc.sbuf_tensor("lhsTA", [128, NA], fp32))
        sg = e(nc.sbuf_tensor("sg", [1, 1], fp32))
        s1 = e(nc.sbuf_tensor("s1", [1, 1], fp32))
        ones_row = e(nc.sbuf_tensor("ones_row", [1, 128], fp32))
        ones_col = e(nc.sbuf_tensor("ones_col", [128, 1], fp32))
        scale_col = e(nc.sbuf_tensor("scale_col", [128, 1], fp32))
        ident01 = e(nc.sbuf_tensor("ident01", [128, 128], fp32))
        # ys[:, 0:128]   scatter-A rows (partitions 0..59)
        # ys[:, 128:256] scatter-B6 rows (partitions 0..9)
        # ys[:, 256:384] scatter-B7 rows (partitions 0..9)
        # ys[:, 0:80]    zero source for the y pre-zero store
        ys = e(nc.sbuf_tensor("ys", [128, 384], fp32))
        dump = e(nc.sbuf_tensor("dump", [128, T], fp32))
        idx_sb = e(nc.sbuf_tensor("idx_sb", [128, 8], i16))

        psc = e(nc.psum_tensor("psc", [128, 1], fp32))
        ptsA = e(nc.psum_tensor("ptsA", [NA, 128], fp32))
        ptsB6 = e(nc.psum_tensor("ptsB6", [NB, 128], fp32))
        ptsB7 = e(nc.psum_tensor("ptsB7", [NB, 128], fp32))

        xld = [e(nc.semaphore(f"xld{k}")) for k in range(BPC + 1)]
        sig_sem = e(nc.semaphore("sig_sem"))
        idx_sem = e(nc.semaphore("idx_sem"))
        zero_sem = e(nc.semaphore("zero_sem"))
        prepA_sem = e(nc.semaphore("prepA_sem"))
        prepB6_sem = e(nc.semaphore("prepB6_sem"))
        prepB7_sem = e(nc.semaphore("prepB7_sem"))
        dmaA_sem = e(nc.semaphore("dmaA_sem"))
        dmaB6_sem = e(nc.semaphore("dmaB6_sem"))
        dmaB7_sem = e(nc.semaphore("dmaB7_sem"))
        dve_sem = e(nc.semaphore("dve_sem"))
        act_sem = e(nc.semaphore("act_sem"))
        pe_sem = e(nc.semaphore("pe_sem"))
        pool_sem = e(nc.semaphore("pool_sem"))

        block = e(nc.Block())

        # DVE milestones (then_inc order): 1 ones_col, 2 ones_row,
        # 3 ys memset, 4 s1, 5 scale_col, 6..11 items 0..5, 12 lhsTA
        # block 5, 13 item7 main partial, 14 item7 tail partial,
        # 15 copyB7.
        MS_YS = 3
        MS_ITEM = {b: 6 + b for b in range(6)}
        MS_COPY5 = 12
        MS_MAIN7 = 13
        MS_TAIL7 = 14
        MS_COPYB7 = 15
        # ACT milestones: 1..5 lhsTA copies 0..4, 6 item6 reduce,
        # 7 copyA, 8 copyB6.
        # PE milestones: 1 psc, 2..7 dummies, 8 trA, 9 trB7a, 10 trB6,
        # 11 trB7b.

        @block.sync
        def _(sync):
            # Chunk order: items 0..5 full, item 7 MAIN, item 6 full,
            # item 7 TAIL.  Item 7's main chunk arrives before item 6 so
            # its DVE reduce runs in the gap before the tail chunk; item
            # 6 (the last full item, reduced on ACT) still lands early
            # enough for its chain to finish off the critical path.
            for b in range(6):
                sync.dma_start(
                    xt[:, b * T : (b + 1) * T], x[b, :, :]
                ).then_inc(xld[b], 16)
            b = BPC - 1
            sync.dma_start(
                xt[:, 6 * T : 7 * T], x[6, :, :]
            ).then_inc(xld[6], 16)
            sync.dma_start(
                xt[:, b * T : b * T + MAIN], x[b, :, 0:MAIN]
            ).then_inc(xld[b], 16)
            sync.dma_start(
                xt[:, b * T + MAIN : (b + 1) * T], x[b, :, MAIN:T]
            ).then_inc(xld[BPC], 16)
            # Pre-zero y (scatter_add accumulates).  The zeros are read
            # from the still-zero ys region; the transfer lands right
            # after the input stream, its sem before any trigger fires.
            sync.wait_ge(dve_sem, MS_YS)
            sync.dma_start(
                y[:, :, :]
                .rearrange("b c f -> (b c f)")
                .rearrange("(p q) -> p q", p=128),
                ys[:, 0 : (BPC * C * F) // 128],
            ).then_inc(zero_sem, 16)
            sync.wait_ge(dmaA_sem, 16)
            sync.wait_ge(dmaB6_sem, 16)
            sync.wait_ge(dmaB7_sem, 16)

        @block.gpsimd
        def _(gpsimd):
            # SWDGE sigma + scatter-index loads keep HWDGE free for the
            # input stream.
            gpsimd.dma_start(sg[:, :], sig[:, :]).then_inc(sig_sem, 16)
            gpsimd.dma_start(idx_sb[:, :], idx[:, :]).then_inc(idx_sem, 16)
            # Prepared scatters; trigger_dma fires them in FIFO order
            # (A, then B6, then B7).  dma_scatter_add lives in the 'mlp'
            # Q7 ucode library.
            gpsimd.wait_ge(idx_sem, 16)
            gpsimd.load_library(library_config.mlp)
            # Prep FIFO = trigger order: A, B6, B7 (matching the order
            # their source rows complete).
            gpsimd.dma_scatter_add(
                out_ap=yrows[0:NA, :],
                in_ap=ys[:, 0:128].rearrange("p (s e) -> p s e", s=1),
                idxs_ap=idx_sb[:, 0:4],
                num_idxs=NA,
                num_idxs_reg=NA,
                elem_size=128,
                prepare_only=True,
                sem=dmaA_sem,
            ).then_inc(prepA_sem, 1)
            gpsimd.dma_scatter_add(
                out_ap=yrows[NA : NA + NB, :],
                in_ap=ys[:, 128:256].rearrange("p (s e) -> p s e", s=1),
                idxs_ap=idx_sb[:, 4:5],
                num_idxs=NB,
                num_idxs_reg=NB,
                elem_size=128,
                prepare_only=True,
                sem=dmaB6_sem,
            ).then_inc(prepB6_sem, 1)
            gpsimd.dma_scatter_add(
                out_ap=yrows[NA + NB : NA + 2 * NB, :],
                in_ap=ys[:, 256:384].rearrange("p (s e) -> p s e", s=1),
                idxs_ap=idx_sb[:, 4:5],
                num_idxs=NB,
                num_idxs_reg=NB,
                elem_size=128,
                prepare_only=True,
                sem=dmaB7_sem,
            ).then_inc(prepB7_sem, 1)
            # 0/1 identity for the PE transposes.
            gpsimd.wait_ge(dve_sem, 1)  # ones_col
            gpsimd.affine_select(
                out=ident01[:, :],
                in_=ones_col[:, :].broadcast_to((128, 128)),
                compare_op=mybir.AluOpType.is_equal,
                fill=0.0,
                base=0,
                pattern=[[-1, 128]],
                channel_multiplier=1,
            ).then_inc(pool_sem, 1)  # ident01 milestone: pool_sem >= 1
            # Fire each scatter as its source rows land: A, B6, B7.
            gpsimd.wait_ge(prepA_sem, 1)
            gpsimd.wait_ge(zero_sem, 16)
            gpsimd.wait_ge(act_sem, 7)  # copyA
            gpsimd.trigger_dma(count=1)
            gpsimd.wait_ge(prepB6_sem, 1)
            gpsimd.wait_ge(act_sem, 8)  # copyB6
            gpsimd.trigger_dma(count=1)
            gpsimd.wait_ge(prepB7_sem, 1)
            gpsimd.wait_ge(dve_sem, MS_COPYB7)
            gpsimd.trigger_dma(count=1)

        @block.vector
        def _(vector):
            vector.memset(ones_col[:, :], 1.0).then_inc(dve_sem, 1)
            vector.memset(ones_row[:, :], 1.0).then_inc(dve_sem, 1)
            vector.memset(ys[:, :], 0.0).then_inc(dve_sem, 1)
            vector.wait_ge(sig_sem, 16)
            # s1 = sigma/T + 1/T = (1+sigma)/T
            vector.tensor_scalar(
                out=s1[:, :],
                in0=sg[:, :],
                scalar1=1.0 / T,
                scalar2=1.0 / T,
                op0=mybir.AluOpType.mult,
                op1=mybir.AluOpType.add,
            ).then_inc(dve_sem, 1)
            vector.wait_ge(pe_sem, 1)  # psc ready
            vector.tensor_copy(scale_col[:, :], psc[:, :]).then_inc(dve_sem, 1)
            # Items 0..5 full reduces (item 6 runs on ACT so the DVE is
            # free for item 7's two chunk reduces at the stream tail).
            for b in range(6):
                vector.wait_ge(xld[b], 16)
                vector.reduce_sum(
                    out=sums[:, b : b + 1],
                    in_=xt[:, b * T : (b + 1) * T],
                    axis=mybir.AxisListType.X,
                ).then_inc(dve_sem, 1)
            # Item 5's lhsTA block on DVE (fits the idle gap before the
            # item 7 reduces; ACT is busy with item 6 then).
            vector.wait_ge(dve_sem, MS_ITEM[5])
            vector.tensor_copy(
                lhsTA[:, 5 * C : 6 * C],
                sums[:, 5:6].broadcast_to((128, C)),
            ).then_inc(dve_sem, 1)
            # Item 7 chunk partials into separate columns; the PE merges
            # them via two accumulating transposes, so neither reduce
            # depends on the other.
            b = BPC - 1
            vector.wait_ge(xld[b], 16)
            vector.reduce_sum(
                out=sums[:, 8:9],
                in_=xt[:, b * T : b * T + MAIN],
                axis=mybir.AxisListType.X,
            ).then_inc(dve_sem, 1)
            vector.wait_ge(xld[BPC], 16)
            vector.reduce_sum(
                out=sums[:, 9:10],
                in_=xt[:, b * T + MAIN : (b + 1) * T],
                axis=mybir.AxisListType.X,
            ).then_inc(dve_sem, 1)
            # Scaled PSUM -> SBUF copy of item 7's rows, right after
            # the tail transpose (DVE is idle post-reduce).
            vector.wait_ge(pe_sem, 11)  # trB7b done
            vector.tensor_tensor(
                out=ys[0:NB, 256:384],
                in0=ptsB7[:, :],
                in1=scale_col[0:NB, :].broadcast_to((NB, 128)),
                op=mybir.AluOpType.mult,
            ).then_inc(dve_sem, 1)

        @block.tensor
        def _(tensor):
            tensor.wait_ge(dve_sem, 4)  # ones_row + s1
            tensor.matmul(
                psc[:, :], ones_row[:, :], s1[:, :], start=True, stop=True
            ).then_inc(pe_sem, 1)
            # Dummy matmuls paced by the per-item reduces keep the PE
            # p-state ramp alive so the tail transposes run at full
            # clock (2.4 GHz after >3 us of busy history).
            for k in range(6):
                tensor.wait_ge(dve_sem, MS_ITEM[k])
                tensor.matmul(
                    psc[0:1, :],
                    ones_row[:, 0:1],
                    s1[:, :],
                    start=True,
                    stop=True,
                ).then_inc(pe_sem, 1)
            # Transpose A first: its deps (lhsTA blocks + ident01) are
            # ready well before the item 6/7 sums.
            tensor.wait_ge(act_sem, 5)  # lhsTA blocks 0..4
            tensor.wait_ge(dve_sem, MS_COPY5)  # lhsTA block 5
            tensor.wait_ge(pool_sem, 1)  # ident01
            tensor.transpose(
                ptsA[:, :], lhsTA[:, :], ident01[:, :]
            ).then_inc(pe_sem, 1)
            # Item 7 main-chunk transpose (PSUM group opens).
            tensor.wait_ge(dve_sem, MS_MAIN7)
            tensor.matmul(
                ptsB7[:, :],
                sums[:, 8:9].broadcast_to((128, NB)),
                ident01[:, :],
                is_transpose=True,
                start=True,
                stop=False,
            ).then_inc(pe_sem, 1)
            # Transpose B6: [128, 10] bcast of sums col 6 -> ptsB6.
            tensor.wait_ge(act_sem, 6)  # item 6 reduce (ACT)
            tensor.transpose(
                ptsB6[:, :],
                sums[:, 6:7].broadcast_to((128, NB)),
                ident01[:, :],
            ).then_inc(pe_sem, 1)
            # Item 7 tail-chunk transpose (PSUM group closes).
            tensor.wait_ge(dve_sem, MS_TAIL7)
            tensor.matmul(
                ptsB7[:, :],
                sums[:, 9:10].broadcast_to((128, NB)),
                ident01[:, :],
                is_transpose=True,
                start=False,
                stop=True,
            ).then_inc(pe_sem, 1)

        @block.scalar
        def _(scalar):
            # lhsTA blocks for items 0..4, paced by the DVE reduces —
            # all hidden in the stream (block 5 is done on DVE).
            for b in range(5):
                scalar.wait_ge(dve_sem, MS_ITEM[b])
                scalar.activation(
                    out=lhsTA[:, b * C : (b + 1) * C],
                    in_=sums[:, b : b + 1].broadcast_to((128, C)),
                    func=mybir.ActivationFunctionType.Copy,
                ).then_inc(act_sem, 1)
            # Item 6 reduce on ACT (activation accum_out) so DVE is
            # free for item 7's two chunk reduces.
            b = 6
            scalar.wait_ge(xld[b], 16)
            scalar.activation(
                out=dump[:, :],
                in_=xt[:, b * T : (b + 1) * T],
                func=mybir.ActivationFunctionType.Copy,
                accum_out=sums[:, b : b + 1],
            ).then_inc(act_sem, 1)
            # Scaled copies (scale rides the activation): A rows, then
            # item 6 rows.
            scalar.wait_ge(pe_sem, 8)  # trA done
            scalar.wait_ge(zero_sem, 16)  # zero store has read ys
            scalar.activation(
                out=ys[0:NA, 0:128],
                in_=ptsA[:, :],
                func=mybir.ActivationFunctionType.Copy,
                scale=scale_col[0:NA, :],
            ).then_inc(act_sem, 1)
            scalar.wait_ge(pe_sem, 10)  # trB6 done
            scalar.activation(
                out=ys[0:NB, 128:256],
                in_=ptsB6[:, :],
                func=mybir.ActivationFunctionType.Copy,
                scale=scale_col[0:NB, :],
            ).then_inc(act_sem, 1)

    bass.Bass.all_engine_barrier = _orig_barrier
    mybir.codegen_inst_isa_subclasses(nc)
    _NC_CACHE = nc
    return nc


def _make_idx() -> np.ndarray:
    """Scatter token indices, replicated per 16-partition Q7 group.

    Cols 0..3: scatter-A tokens (token 16*s + i -> local y row, identity
    for the first 60 tokens, 0-clamped padding after).  Col 4: scatter-B
    tokens (identity for the first 10).  Values are row offsets local to
    each scatter's out_ap.
    """
    idx = np.zeros((128, 8), dtype=np.int16)
    a = np.arange(64).reshape(4, 16).T  # [i, s] = 16*s + i
    a = np.where(a < 60, a, 0).astype(np.int16)
    b = np.arange(16, dtype=np.int16)
    b = np.where(b < 10, b, 0).astype(np.int16)
    for g in range(8):
        idx[16 * g : 16 * (g + 1), 0:4] = a
        idx[16 * g : 16 * (g + 1), 4] = b
    return idx


def run_spmd(inputs_arr: np.ndarray, sigma_arr: np.ndarray, trace: bool = False):
    """Shard over batch, run on 8 cores, gather. Returns (out, results_obj)."""
    from concourse import bass_utils

    nc = _build_bass()

    x_full = np.ascontiguousarray(np.asarray(inputs_arr, dtype=np.float32))
    assert x_full.shape == (B, F, T), x_full.shape
    sig = np.asarray(sigma_arr, dtype=np.float32).reshape(1, 1)
    idx = _make_idx()

    in_maps = [
        {"x": x_full[k * BPC : (k + 1) * BPC], "sig": sig, "idx": idx}
        for k in range(N_CORES)
    ]
    res = bass_utils.run_bass_kernel_spmd(
        nc, in_maps, core_ids=list(range(N_CORES)), trace=trace
    )
    out = np.concatenate([r["y"] for r in res.results], axis=0)
    return out, res


def kernel(**inputs) -> np.ndarray:
    out, _ = run_spmd(inputs["inputs"], inputs["sigma"])
    return out
